# revision 31
# baseline (speedup 1.0000x reference)
"""DDiT block (adaLN-modulated transformer block) on 8 Trainium2 NeuronCores.

Sharding: tokens split 8 ways (2 batches x 4 sequence chunks of 512 tokens).
Activations kept feature-major ([feature, token]) on-chip. k/v all-gathered
within each batch group of 4 cores in two stages (heads 0-5, then heads
6-11). adaLN modulation (a [B,6H] vector that depends only on c) is folded
into per-core constants on the host, like the weight transposes/quant.

Precision: fp32 residual stream and layernorm statistics; bf16 for LN sums,
rotary and broadcast matmuls; fp8e4 (DoubleRow where the contraction allows)
for the qkv projection, attention scores, attn@v, attn_out, mlp1 and mlp2
matmuls. Scales are powers of two folded into downstream constants, so
dequantization is exact.
"""
import os
import sys

for _p in ("/opt/trn_rl_repo", "/root/.axon_site/_ro/trn_rl_repo"):
    if os.path.isdir(_p) and _p not in sys.path:
        sys.path.append(_p)

import numpy as np
import ml_dtypes

import concourse.bass as bass
import concourse.mybir as mybir
import concourse.tile as tile
from concourse.bass_utils import run_bass_kernel_spmd
from concourse.vector_clock import ScopedClock

BF16 = ml_dtypes.bfloat16
FP8 = ml_dtypes.float8_e4m3
F32 = np.float32

B, S, H, NH, D, Fd = 2, 2048, 768, 12, 64, 3072
P = 128
NCORES = 8
TOK = S // 4            # 512 tokens per core
KT = H // P             # 6 feature tiles of H
FT = Fd // P            # 24 feature tiles of F
VW = D + 4              # 68: v + "32" column (softmax denom) + zero pad,
                        # so the DoubleRow weight AP's chunk stride
                        # (NH*VW = 816 bytes) is 16-byte aligned
EPS = 1e-5

# fp8 scale factors (powers of two; dequant folded into constants)
SX = 16.0               # xm / xm2 (modulated LN outputs)
SW = 256.0              # all fp8 weights
SQK = 8.0               # rotated q / k
SV = 32.0               # v (also the value of the denominator column)
SO = 16.0               # normalized attention output o
DQ = 1.0 / (SX * SW)    # dequant of fp8 matmul psums (2^-12)
DQ2 = 1.0 / SW          # dequant of the mlp2 psum (hdn is unscaled fp8)
EXP_BIAS = float(-np.log(SQK))  # exp(logit)-scale folded via bias
EXP_SCALE = 1.0 / (D * SQK * SQK) * 8.0  # psum = 64*dot*64 -> logit=dot/8

AF = mybir.ActivationFunctionType
ALU = mybir.AluOpType
DT = mybir.dt
PM = mybir.MatmulPerfMode

RG_BATCH = [[0, 1, 2, 3], [4, 5, 6, 7]]

KBN = P * TOK           # elements of one k tile in an AG buffer
VBN = P * 6 * VW        # elements of one v half-toktile in an AG buffer
AGN = 3 * KBN + 4 * VBN  # per-stage AG payload (fp8 elements)


def _patch_tile_drain():
    """The walrus build here allows at most one semaphore wait on SP
    control instructions; TileContext's exit drain attaches several.
    Split them one-per-NOP."""
    if getattr(tile.TileContext, "_ant_drain_patched", False):
        return

    def _split_multiwaits(nc):
        count = 0
        for f in nc.m.functions:
            for bb in f.blocks:
                insts = bb.instructions
                new = []
                for ins in insts:
                    si = getattr(ins, "sync_info", None)
                    if si is not None and si.on_wait and len(si.on_wait) > 1:
                        waits = list(si.on_wait)
                        si.on_wait = [waits[-1]]
                        for w in waits[:-1]:
                            count += 1
                            nop = mybir.InstNoOp(
                                name=f"antw_{count}_{ins.name}",
                                ins=[], outs=[])
                            nop.engine = ins.engine
                            nop.sync_info = mybir.SyncInfo(
                                on_update=[], on_wait=[w])
                            nc.register_instruction(nop, overwrite=True)
                            new.append(nop)
                    new.append(ins)
                bb.instructions = new

    def _drain_and_barrier(self, tick_clock, wait_clock):
        _split_multiwaits(self.nc)
        drain_inst = self.nc.sync.drain()
        wait_clock.add_sem_waits(
            drain_inst.ins, ScopedClock({None: tick_clock.global_clock})
        )
        si = drain_inst.ins.sync_info
        waits = list(si.on_wait)
        si.on_wait = []
        for w in waits:
            nop = self.nc.sync.nop(nofuse=True, hint="drain_extra_waits")
            nop.ins.sync_info = mybir.SyncInfo(on_update=[], on_wait=[w])
        self.nc.all_engine_barrier()
        popped = self.nc._tile_sem_poison_stack.pop()
        assert popped is self._sem_poison
        self.nc.clear_and_free_semaphores(list(self.sems.allocated().values()))
        self.nc.all_engine_barrier()

    tile.TileContext._drain_and_barrier = _drain_and_barrier
    tile.TileContext._ant_drain_patched = True


def build():
    _patch_tile_drain()
    nc = bass.Bass(num_devices=NCORES)

    def din(name, shape, dt):
        return nc.dram_tensor(name, shape, dt, kind="ExternalInput")

    xT = din("xT", [KT, P, TOK], DT.float32)
    qkvw8 = din("qkvw8", [KT, P, 3 * H], DT.float8e4)
    attnw8 = din("attnw8", [P, 6 * H], DT.float8e4)
    w18 = din("w18", [KT, P, Fd], DT.float8e4)
    w2T = din("w2T", [KT, FT, P, P], DT.float8e4)
    b1 = din("b1", [P, FT], DT.float32)
    n1wA = din("n1wA", [P, KT], DT.float32)   # 16*norm1_w*(1+scale_msa)
    n2wA = din("n2wA", [P, KT], DT.float32)   # 16*norm2_w*(1+scale_mlp)
    sh1 = din("sh1", [P, KT], DT.float32)     # 16*shift_msa
    sh2 = din("sh2", [P, KT], DT.float32)     # 16*shift_mlp
    g1 = din("g1", [P, KT], DT.float32)       # gate_msa*DQ
    g2dq = din("g2dq", [P, KT], DT.float32)   # gate_mlp*DQ2
    b2g = din("b2g", [P, KT], DT.float32)     # mlp_b2*gate_mlp
    cos8 = din("cos8", [P, TOK], DT.bfloat16)   # cos * SQK/(SX*SW)
    sin8 = din("sin8", [P, TOK], DT.bfloat16)   # sin * SQK/(SX*SW)
    rotp = din("rotp", [P, P], DT.bfloat16)
    onesb = din("onesb", [P, P], DT.bfloat16)
    # bsel[r, s*128 + p]: broadcast-select stationary. Head denominators
    # live at partition 32*(h%4); a head pair tt uses slots (0,32) when
    # tt is even (s=0) and (64,96) when odd (s=1); even head -> cols 0-63.
    bsel = din("bsel", [P, 2 * P], DT.bfloat16)

    outT = nc.dram_tensor("outT", [KT, P, TOK], DT.float32,
                          kind="ExternalOutput")

    with tile.TileContext(nc) as tc:
        with tc.tile_pool(name="sb", bufs=1) as sb, \
             tc.tile_pool(name="ps", bufs=1, space="PSUM") as ps, \
             tc.tile_pool(name="dr", bufs=1, space="DRAM") as dr:
            _body(nc, sb, ps, dr, locals())
    return nc


def _body(nc, sb, ps, dr, t):
    xT, qkvw8, attnw8, w18, w2T = t["xT"], t["qkvw8"], t["attnw8"], t["w18"], t["w2T"]
    b1, n1wA, n2wA = t["b1"], t["n1wA"], t["n2wA"]
    sh1, sh2, g1, g2dq, b2g = t["sh1"], t["sh2"], t["g1"], t["g2dq"], t["b2g"]
    cos8, sin8, rotp, onesb, bsel = t["cos8"], t["sin8"], t["rotp"], t["onesb"], t["bsel"]
    outT = t["outT"]

    # x first: everything up to the first AllGather hangs off it
    x_sb = sb.tile([P, KT, TOK], DT.float32)
    for k in range(KT):
        nc.sync.dma_start(x_sb[:, k, :], xT[k])

    # ================= constants ===================================
    zero_c = sb.tile([P, 1], DT.float32)
    nc.vector.memset(zero_c[:], 0.0)
    nc.const_aps.aps[(DT.float32, 0.0)] = zero_c[:]
    eps_c = sb.tile([P, 1], DT.float32)
    nc.vector.memset(eps_c[:], EPS)
    nc.const_aps.aps[(DT.float32, EPS)] = eps_c[:]
    expb_c = sb.tile([P, 1], DT.float32)
    nc.vector.memset(expb_c[:], EXP_BIAS)
    nc.const_aps.aps[(DT.float32, EXP_BIAS)] = expb_c[:]

    # ================= warm-up collective ==========================
    # First collective on the CC stream pays a large arming cost that
    # includes waiting for all peers; fire a 16B dummy immediately so
    # that cost overlaps the compute prologue instead of the k/v AG.
    dummy_sb = sb.tile([1, 4], DT.float32)
    nc.vector.memset(dummy_sb[:], 0.0)
    dummy_in = dr.tile([4], DT.float32)
    dummy_out = dr.tile([4, 4], DT.float32)
    nc.sync.dma_start(dummy_in[:].rearrange("(a b) -> a b", a=1), dummy_sb[:])
    nc.gpsimd.collective_compute(
        "AllGather", ALU.bypass, replica_groups=RG_BATCH,
        ins=[dummy_in[:].opt()], outs=[dummy_out[:].opt()])

    # ================= DMA loads (priority order) ==================
    onesb_sb = sb.tile([P, P], DT.bfloat16)
    nc.sync.dma_start(onesb_sb[:], onesb[:])
    bsel_sb = sb.tile([P, 2 * P], DT.bfloat16)
    nc.sync.dma_start(bsel_sb[:], bsel[:])
    rotp_sb = sb.tile([P, P], DT.bfloat16)
    nc.sync.dma_start(rotp_sb[:], rotp[:])
    cos_sb = sb.tile([P, TOK], DT.bfloat16)
    nc.sync.dma_start(cos_sb[:], cos8[:])
    sin_sb = sb.tile([P, TOK], DT.bfloat16)
    nc.sync.dma_start(sin_sb[:], sin8[:])
    n1wA_sb = sb.tile([P, KT], DT.float32)
    nc.sync.dma_start(n1wA_sb[:], n1wA[:])
    n2wA_sb = sb.tile([P, KT], DT.float32)
    nc.sync.dma_start(n2wA_sb[:], n2wA[:])
    sh1_sb = sb.tile([P, KT], DT.float32)
    nc.sync.dma_start(sh1_sb[:], sh1[:])
    sh2_sb = sb.tile([P, KT], DT.float32)
    nc.sync.dma_start(sh2_sb[:], sh2[:])
    g1_sb = sb.tile([P, KT], DT.float32)
    nc.sync.dma_start(g1_sb[:], g1[:])
    g2dq_sb = sb.tile([P, KT], DT.float32)
    nc.sync.dma_start(g2dq_sb[:], g2dq[:])
    b2g_sb = sb.tile([P, KT], DT.float32)
    nc.sync.dma_start(b2g_sb[:], b2g[:])

    # qkv weights: k columns first (gate the first AG)
    qkvw_sb = sb.tile([P, KT, 3 * H], DT.float8e4)
    for k in range(KT):
        nc.sync.dma_start(qkvw_sb[:, k, H:2 * H], qkvw8[k][:, H:2 * H])
    for k in range(KT):
        nc.sync.dma_start(qkvw_sb[:, k, 2 * H:3 * H], qkvw8[k][:, 2 * H:3 * H])
    for k in range(KT):
        nc.sync.dma_start(qkvw_sb[:, k, 0:H], qkvw8[k][:, 0:H])
    attnw_sb = sb.tile([P, 6, H], DT.float8e4)
    nc.sync.dma_start(attnw_sb[:].rearrange("p a b -> p (a b)"), attnw8[:])
    b1_sb = sb.tile([P, FT], DT.float32)
    nc.sync.dma_start(b1_sb[:], b1[:])
    w1_sb = sb.tile([P, KT, Fd], DT.float8e4)
    for k in range(KT):
        nc.sync.dma_start(w1_sb[:, k, :], w18[k])

    f32s = dict(tag="f32s", bufs=3)

    def layer_norm(src_sb, xb, A_tile, sh_tile, xm_out):
        """src [128,KT,TOK] f32 (+ its bf16 copy xb) ->
        xm_out [128,KT,TOK] fp8e4 = 16*(modulated LN output)."""
        s_ps = ps.tile([1, TOK], DT.float32, tag="duo", bufs=3, name="s_ps")
        q_ps = ps.tile([1, TOK], DT.float32, tag="duo", bufs=3, name="q_ps")
        for tt in range(KT):
            nc.gpsimd.tensor_copy(xb[:, tt, :], src_sb[:, tt, :])
            xsq = sb.tile([P, TOK], DT.bfloat16, tag="sq", bufs=2, name="xsq")
            nc.gpsimd.tensor_tensor(xsq[:], xb[:, tt, :], xb[:, tt, :],
                                    ALU.mult)
            nc.tensor.matmul(s_ps[:], onesb_sb[:, 0:1], xb[:, tt, :],
                             start=(tt == 0), stop=(tt == KT - 1))
            nc.tensor.matmul(q_ps[:], onesb_sb[:, 0:1], xsq[:],
                             start=(tt == 0), stop=(tt == KT - 1))
        sa = sb.tile([1, TOK], DT.float32, tag="st_a", bufs=1, name="sa")
        sb2 = sb.tile([1, TOK], DT.float32, tag="st_b", bufs=1, name="sb2")
        sc_ = sb.tile([1, TOK], DT.float32, tag="st_c", bufs=1, name="sc_")
        nc.vector.tensor_scalar(sa[:], s_ps[:], 1.0 / H, None, ALU.mult)
        nc.vector.tensor_scalar(sb2[:], q_ps[:], 1.0 / H, None, ALU.mult)
        nc.vector.tensor_tensor(sc_[:], sa[:], sa[:], ALU.mult)
        nc.vector.tensor_tensor(sb2[:], sb2[:], sc_[:], ALU.subtract)
        nc.scalar.activation(sc_[:], sb2[:], AF.Sqrt, bias=EPS)
        nc.vector.reciprocal(sb2[:], sc_[:])   # sb2 = rstd (f32)
        rstd16 = sb.tile([1, TOK], DT.bfloat16, tag="st_d", bufs=1,
                         name="rstd16")
        nc.vector.tensor_scalar(rstd16[:], sb2[:], 1.0, None, ALU.mult)
        mr16 = sb.tile([1, TOK], DT.bfloat16, tag="st_e", bufs=1, name="mr16")
        nc.vector.tensor_tensor(mr16[:], sa[:], sb2[:], ALU.mult)
        rstd_ps = ps.tile([P, TOK], DT.float32, tag="duo", bufs=3,
                          name="rstd_ps")
        mr_ps = ps.tile([P, TOK], DT.float32, tag="duo", bufs=3,
                        name="mr_ps")
        nc.tensor.matmul(rstd_ps[:], onesb_sb[0:1, :], rstd16[:],
                         start=True, stop=True)
        nc.tensor.matmul(mr_ps[:], onesb_sb[0:1, :], mr16[:],
                         start=True, stop=True)
        for tt in range(KT):
            t1 = sb.tile([P, TOK], DT.float32, **f32s, name="t1")
            nc.vector.tensor_tensor(t1[:], src_sb[:, tt, :], rstd_ps[:],
                                    ALU.mult)
            nc.vector.tensor_tensor(t1[:], t1[:], mr_ps[:], ALU.subtract)
            nc.vector.tensor_scalar(
                xm_out[:, tt, :], t1[:], A_tile[:, tt:tt + 1],
                sh_tile[:, tt:tt + 1], ALU.mult, ALU.add)

    # ================= LN1 + qkv ===================================
    xb_sb = sb.tile([P, KT, TOK], DT.bfloat16, tag="xb")
    xm_sb = sb.tile([P, KT, TOK], DT.float8e4, tag="xm")
    layer_norm(x_sb, xb_sb, n1wA_sb, sh1_sb, xm_sb)

    agA_in = dr.tile([AGN], DT.float8e4)
    agA_out = dr.tile([4, AGN], DT.float8e4)
    agB_in = dr.tile([AGN], DT.float8e4)
    agB_out = dr.tile([4, AGN], DT.float8e4)

    kfull_sb = sb.tile([P, KT, S], DT.float8e4)
    vfull_sb = sb.tile([P, 16, NH * VW], DT.float8e4)

    def qk_tile(m, dest, dest2=None, ve=None):
        """Feature tile m of the qkv projection + rotary -> fp8 (x8).
        ve: engine for the SBUF-only rotary combine (Pool for k tiles,
        emitted before the collective triggers on the Pool queue; DVE
        for q tiles). The PSUM-reading multiply always runs on DVE."""
        acc = ps.tile([P, TOK], DT.float32, tag="duo", bufs=3, name="qk_acc")
        for k in range(0, KT, 2):
            nc.tensor.matmul(acc[:], qkvw_sb[:, k:k + 2, m * P:(m + 1) * P],
                             xm_sb[:, k:k + 2, :],
                             start=(k == 0), stop=(k == KT - 2),
                             perf_mode=PM.DoubleRow)
        pre = sb.tile([P, TOK], DT.bfloat16, tag="qpre", bufs=2, name="pre")
        if m % 2:
            nc.vector.tensor_copy(pre[:], acc[:])
        else:
            nc.scalar.copy(pre[:], acc[:])
        rot = ps.tile([P, TOK], DT.float32, tag="duo", bufs=3, name="rot")
        nc.tensor.matmul(rot[:], rotp_sb[:], pre[:], start=True, stop=True)
        r1 = sb.tile([P, TOK], DT.bfloat16, tag="rr1", bufs=2, name="r1")
        nc.gpsimd.tensor_tensor(r1[:], pre[:], cos_sb[:], ALU.mult)
        r2 = sb.tile([P, TOK], DT.bfloat16, tag="rr2", bufs=2, name="r2")
        nc.vector.tensor_tensor(r2[:], rot[:], sin_sb[:], ALU.mult)
        if dest2 is None:
            ve.tensor_tensor(dest, r1[:], r2[:], ALU.add)
        else:
            ve.tensor_tensor(dest, r1[0:D, :], r2[0:D, :], ALU.add)
            ve.tensor_tensor(dest2, r1[D:P, :], r2[D:P, :], ALU.add)

    vaug_sb = sb.tile([P, 4, NH * VW], DT.float8e4)
    for tt in range(4):
        nc.vector.memset(
            vaug_sb[:, tt, :].rearrange("p (h w) -> p h w", w=VW)[:, :, D:D + 1],
            SV)
        nc.vector.memset(
            vaug_sb[:, tt, :].rearrange("p (h w) -> p h w", w=VW)[:, :, D + 1:VW],
            0.0)

    def v_tile(tt, half):
        acc = ps.tile([P, 6 * D], DT.float32, tag="duo", bufs=3,
                      name="v_acc")
        for k in range(0, KT, 2):
            nc.tensor.matmul(
                acc[:], xm_sb[:, k:k + 2, tt * P:(tt + 1) * P],
                qkvw_sb[:, k:k + 2, 2 * H + half * 6 * D:
                        2 * H + (half + 1) * 6 * D],
                start=(k == 0), stop=(k == KT - 2),
                perf_mode=PM.DoubleRow)
        nc.vector.tensor_scalar(
            vaug_sb[:, tt, :]
            .rearrange("p (h w) -> p h w", w=VW)[:, half * 6:(half + 1) * 6, 0:D],
            acc[:].rearrange("p (h d) -> p h d", d=D), SV * DQ, None, ALU.mult)

    def ag(in_t, out_t):
        nc.gpsimd.collective_compute(
            "AllGather", ALU.bypass, replica_groups=RG_BATCH,
            ins=[in_t[:].opt()], outs=[out_t[:].opt()])

    # ---- stage A inputs: k feature tiles 0-2 + v heads 0-5 --------
    for mm_ in range(3):
        kt_t = sb.tile([P, TOK], DT.float8e4, tag="ktmp", bufs=2,
                       name=f"ktmpA_{mm_}")
        qk_tile(KT + mm_, kt_t[:], ve=nc.gpsimd)
        nc.sync.dma_start(
            agA_in[mm_ * KBN:(mm_ + 1) * KBN].rearrange("(p s) -> p s", p=P),
            kt_t[:])
    for tt in range(4):
        v_tile(tt, 0)
        nc.sync.dma_start(
            agA_in[3 * KBN + tt * VBN:3 * KBN + (tt + 1) * VBN]
            .rearrange("(p w) -> p w", p=P),
            vaug_sb[:, tt, 0:6 * VW])
    ag(agA_in, agA_out)

    # ---- stage B inputs: k feature tiles 3-5 + v heads 6-11 -------
    for mm_ in range(3):
        kt_t = sb.tile([P, TOK], DT.float8e4, tag="ktmp", bufs=2,
                       name=f"ktmpB_{mm_}")
        qk_tile(KT + 3 + mm_, kt_t[:], ve=nc.gpsimd)
        nc.sync.dma_start(
            agB_in[mm_ * KBN:(mm_ + 1) * KBN].rearrange("(p s) -> p s", p=P),
            kt_t[:])
    for tt in range(4):
        v_tile(tt, 1)
        nc.sync.dma_start(
            agB_in[3 * KBN + tt * VBN:3 * KBN + (tt + 1) * VBN]
            .rearrange("(p w) -> p w", p=P),
            vaug_sb[:, tt, 6 * VW:12 * VW])
    ag(agB_in, agB_out)

    # ---- unpack gathered k/v (ahead of q: fires the moment AGs land)
    for r in range(4):
        nc.sync.dma_start(
            kfull_sb[:, 0:3, r * TOK:(r + 1) * TOK],
            agA_out[r, 0:3 * KBN].rearrange("(k p s) -> p k s", p=P, s=TOK))
        nc.sync.dma_start(
            vfull_sb[:, 4 * r:4 * (r + 1), 0:6 * VW],
            agA_out[r, 3 * KBN:].rearrange("(t p w) -> p t w", p=P, w=6 * VW))
    for r in range(4):
        nc.sync.dma_start(
            kfull_sb[:, 3:6, r * TOK:(r + 1) * TOK],
            agB_out[r, 0:3 * KBN].rearrange("(k p s) -> p k s", p=P, s=TOK))
        nc.sync.dma_start(
            vfull_sb[:, 4 * r:4 * (r + 1), 6 * VW:12 * VW],
            agB_out[r, 3 * KBN:].rearrange("(t p w) -> p t w", p=P, w=6 * VW))

    # ---- mlp2 weights prefetched during attention -----------------
    w2all = sb.tile([P, KT, FT, P], DT.float8e4, name="w2all")
    for m in range(KT):
        nc.sync.dma_start(w2all[:, m, :, :], w2T[m].rearrange("k p q -> p k q"))

    # ---- q while the all-gathers are in flight --------------------
    q_sb = sb.tile([P, KT, TOK], DT.float8e4, name="q8")
    for m in range(KT):
        qk_tile(m, q_sb[:, m, :], ve=nc.vector)

    # ---- PE warm-up filler ----------------------------------------
    # The AG-A wait would otherwise idle the PE past the HAM MID window,
    # so attention would start clock-throttled to 1.2 GHz and the first
    # idle-triggered re-throttle tends to stick. Burn the wait with
    # discarded matmuls that keep the activity monitor in the 8/8 state.
    for i in range(30):
        fl = ps.tile([P, TOK], DT.float32, tag="duo", bufs=3,
                     name=f"fill_{i}")
        nc.tensor.matmul(fl[:], rotp_sb[:], cos_sb[:], start=True, stop=True)

    # ================= attention ===================================
    o8_sb = sb.tile([P, 6, TOK], DT.float8e4, name="o8")
    o_raw = sb.tile([P, 6, TOK], DT.bfloat16, name="o_raw")
    # head h's denominator at partition 32*(h%4), column group h//4; the
    # unused partitions are preset to 1.0 so the batched reciprocal and the
    # broadcast matmul never see junk (0 * NaN).
    denoms = sb.tile([P, 3, TOK], DT.float32, name="denoms")
    nc.vector.memset(denoms[:].rearrange("p a s -> p (a s)"), 1.0)
    rd_f = sb.tile([P, 3, TOK], DT.float32, name="rd_f")
    rd16 = sb.tile([P, 3, TOK], DT.bfloat16, name="rd16")

    # bf16 Schraudolph exp on DVE: bits16 = int16((z*log2e + 127-sigma)*2^7)
    # for z = EXP_SCALE*psum + EXP_BIAS; bitcast to bf16 ~= exp(z) (+-3%),
    # then clamp to the fp8 max during the fp8 cast (DVE cast doesn't
    # saturate: 480 -> inf -> NaN in the av matmul otherwise).
    LOG2E = float(np.log2(np.e))
    SCH_C1 = EXP_SCALE * LOG2E * 128.0
    SCH_C2 = (EXP_BIAS * LOG2E + 126.94269504) * 128.0

    def emit_scores(h, pair, eq, dve_exp):
        ht = h // 2
        ro = (h % 2) * D
        sc = ps.tile([P, 2 * TOK], DT.float32, tag="duo", bufs=3,
                     name=f"sc_{h}_{pair}")
        for i in range(2):
            kj = 2 * pair + i
            nc.tensor.matmul(
                sc[:, i * TOK:(i + 1) * TOK],
                kfull_sb[ro:ro + D, ht, kj * P:(kj + 1) * P],
                q_sb[ro:ro + D, ht, :],
                start=True, stop=True, tile_position=(ro, 0))
        dst = eq[:, (pair % 2) * 2:(pair % 2) * 2 + 2, :].rearrange(
            "p a s -> p (a s)")
        if dve_exp:
            ei = sb.tile([P, 2 * TOK], DT.int16, tag="eint", bufs=3,
                         name=f"ei_{h}_{pair}")
            nc.vector.tensor_scalar(ei[:], sc[:], SCH_C1, SCH_C2,
                                    ALU.mult, ALU.add)
            nc.vector.tensor_scalar(dst, ei[:].bitcast(DT.bfloat16),
                                    448.0, None, ALU.min)
        else:
            nc.scalar.activation(dst, sc[:], AF.Exp, scale=EXP_SCALE,
                                 bias=EXP_BIAS)

    def emit_av(h, pair, eq, o_ps, start, stop):
        nc.tensor.matmul(
            o_ps[:], vfull_sb[:, 2 * pair:2 * pair + 2, h * VW:(h + 1) * VW],
            eq[:, (pair % 2) * 2:(pair % 2) * 2 + 2, :],
            start=start, stop=stop, perf_mode=PM.DoubleRow)

    def save_head(h, o_ps):
        # raw (unnormalized) o and its denominator row; normalization is
        # batched per 4 heads so the DVE reciprocal runs 3x, not 12x.
        ro = (h % 2) * D
        tt = h // 2
        dslot = 32 * (h % 4)
        if h % 2:
            nc.vector.tensor_copy(o_raw[ro:ro + D, tt, :], o_ps[0:D, :])
            nc.vector.tensor_copy(denoms[dslot:dslot + 1, h // 4, :],
                                  o_ps[D:D + 1, :])
        else:
            nc.scalar.copy(o_raw[ro:ro + D, tt, :], o_ps[0:D, :])
            nc.scalar.copy(denoms[dslot:dslot + 1, h // 4, :],
                           o_ps[D:D + 1, :])

    def recip_chunk(g, c):
        # 1/4 of the batched reciprocal for heads 4g..4g+3; chunked so the
        # DVE queue can interleave attention exps between pieces
        nc.vector.reciprocal(rd_f[:, g, c * 128:(c + 1) * 128],
                             denoms[:, g, c * 128:(c + 1) * 128])
        if c == 3:
            nc.vector.tensor_scalar(rd16[:, g, :], rd_f[:, g, :], SO, None,
                                    ALU.mult)

    def norm_pair(tt):
        # heads 2tt, 2tt+1 -> o8 = o_raw * SO/denom
        rdb = ps.tile([P, TOK], DT.float32, tag="duo", bufs=3,
                      name=f"rdb_{tt}")
        nc.tensor.matmul(rdb[:], bsel_sb[:, (tt % 2) * P:(tt % 2 + 1) * P],
                         rd16[:, tt // 2, :], start=True, stop=True)
        nc.vector.tensor_tensor(o8_sb[:, tt, :], o_raw[:, tt, :],
                                rdb[:], ALU.mult)

    # Software-pipelined head loop: scores run 2 pairs ahead of attn@v so
    # the tensor queue never waits on a just-issued exp; exps are split
    # between the scalar engine (table exp) and DVE (Schraudolph bf16 exp
    # via int16 bit trick + clamped fp8 cast) to keep either from pacing
    # the PE. On heads that also carry the batched-reciprocal DVE work,
    # the DVE gets one pair less.
    eq_map = {}
    o_tiles = {}

    def emit_sc_exp(h, pair):
        if pair % 2 == 0:
            eq_map[(h, pair // 2)] = sb.tile(
                [P, 4, TOK], DT.float8e4, tag="scr4", bufs=6,
                name=f"exp_{h}_{pair // 2}")
        eq = eq_map[(h, pair // 2)]
        carries_recip = h % 4 == 0 and h > 0
        dve_pairs = (1, 4) if carries_recip else (1, 4, 6)
        emit_scores(h, pair, eq, pair in dve_pairs)
        if carries_recip and pair in (2, 3, 5, 7):
            recip_chunk(h // 4 - 1, {2: 0, 3: 1, 5: 2, 7: 3}[pair])

    def emit_av_for(h, pair):
        if pair == 0:
            o_tiles[h] = ps.tile([VW, TOK], DT.float32, tag="oo", bufs=2,
                                 name=f"o_ps_{h}")
        emit_av(h, pair, eq_map[(h, pair // 2)], o_tiles[h],
                start=(pair == 0), stop=(pair == 7))
        if pair == 7:
            for q_ in range(4):
                eq_map.pop((h, q_), None)
            save_head(h, o_tiles.pop(h))
            if h % 4 == 3 and h // 4 >= 1:
                # group g-1's broadcasts: its reciprocal is ~4 heads old,
                # so the rdb matmul never blocks the PE queue
                g = h // 4
                norm_pair(2 * g - 2)
                norm_pair(2 * g - 1)

    pend = []
    for h in range(NH):
        for pair in range(8):
            emit_sc_exp(h, pair)
            pend.append((h, pair))
            if len(pend) > 2:
                emit_av_for(*pend.pop(0))
    while pend:
        emit_av_for(*pend.pop(0))
    for c in range(4):
        recip_chunk(2, c)
    norm_pair(4)
    norm_pair(5)

    # ================= attn_out + residual =========================
    for m in range(KT):
        acc = ps.tile([P, TOK], DT.float32, tag="duo", bufs=3,
                      name=f"ao_{m}")
        for q_ in range(3):
            nc.tensor.matmul(
                acc[:], attnw_sb[:, 2 * q_:2 * q_ + 2, m * P:(m + 1) * P],
                o8_sb[:, 2 * q_:2 * q_ + 2, :],
                start=(q_ == 0), stop=(q_ == 2), perf_mode=PM.DoubleRow)
        tg = sb.tile([P, TOK], DT.float32, **f32s, name="tg")
        nc.vector.tensor_scalar(tg[:], acc[:], g1_sb[:, m:m + 1], None,
                                ALU.mult)
        nc.vector.tensor_tensor(x_sb[:, m, :], tg[:], x_sb[:, m, :], ALU.add)

    # ================= LN2 + MLP ===================================
    xb2_sb = sb.tile([P, KT, TOK], DT.bfloat16, tag="xb", name="xb2")
    xm2_sb = sb.tile([P, KT, TOK], DT.float8e4, tag="xm", name="xm2")
    layer_norm(x_sb, xb2_sb, n2wA_sb, sh2_sb, xm2_sb)

    hdn_tiles = []
    for g in range(KT):
        hq = sb.tile([P, 4, TOK], DT.float8e4, tag="hdn", bufs=6,
                     name=f"hdn_{g}")
        hdn_tiles.append(hq)
        for r in range(4):
            m = g * 4 + r
            acc = ps.tile([P, TOK], DT.float32, tag="duo", bufs=3,
                          name=f"m1_{m}")
            for k in range(0, KT, 2):
                nc.tensor.matmul(acc[:], w1_sb[:, k:k + 2, m * P:(m + 1) * P],
                                 xm2_sb[:, k:k + 2, :],
                                 start=(k == 0), stop=(k == KT - 2),
                                 perf_mode=PM.DoubleRow)
            nc.scalar.activation(hq[:, r, :], acc[:], AF.Gelu_apprx_tanh,
                                 bias=b1_sb[:, m:m + 1], scale=DQ)

    for m in range(KT):
        acc = ps.tile([P, TOK], DT.float32, tag="duo", bufs=3,
                      name=f"m2_{m}")
        for k in range(0, FT, 2):
            nc.tensor.matmul(acc[:], w2all[:, m, k:k + 2, :],
                             hdn_tiles[k // 4][:, k % 4:k % 4 + 2, :],
                             start=(k == 0), stop=(k == FT - 2),
                             perf_mode=PM.DoubleRow)
        tg = sb.tile([P, TOK], DT.float32, **f32s, name="tg2")
        nc.vector.tensor_scalar(tg[:], acc[:], g2dq_sb[:, m:m + 1],
                                b2g_sb[:, m:m + 1], ALU.mult, ALU.add)
        nc.vector.tensor_tensor(x_sb[:, m, :], tg[:], x_sb[:, m, :], ALU.add)
        nc.sync.dma_start(outT[m], x_sb[:, m, :])


_CACHE = {}


def _get_nc():
    if "nc" not in _CACHE:
        _CACHE["nc"] = build()
    return _CACHE["nc"]


def _rot_perm():
    blk = np.zeros((D, D), F32)
    for i in range(32):
        blk[i, i + 32] = 1.0
    for i in range(32, D):
        blk[i, i - 32] = -1.0
    out = np.zeros((P, P), F32)
    out[0:D, 0:D] = blk
    out[D:P, D:P] = blk
    return out


def _q8(w, s):
    return np.clip(np.asarray(w, F32) * s, -240.0, 240.0).astype(FP8)


def _prep_core_inputs(inputs, core):
    b, j = divmod(core, 4)
    sl = slice(j * TOK, (j + 1) * TOK)
    x = np.asarray(inputs["x"], F32)
    qkv_w = np.asarray(inputs["qkv_w"], F32)
    attn_out_w = np.asarray(inputs["attn_out_w"], F32)
    mlp_w1 = np.asarray(inputs["mlp_w1"], F32)
    mlp_w2 = np.asarray(inputs["mlp_w2"], F32)
    ada_w = np.asarray(inputs["ada_w"], F32)
    ada_b = np.asarray(inputs["ada_b"], F32)
    cc = np.asarray(inputs["c"], F32)
    cos = np.asarray(inputs["cos"], F32)
    sin = np.asarray(inputs["sin"], F32)

    def fm(vec):  # [n*128] -> [128, n] feature-major
        return np.ascontiguousarray(vec.reshape(-1, P).T, dtype=F32)

    # adaLN modulation: a [6H] vector depending only on c[b]; fold on host
    mods = cc[b] @ ada_w.T + ada_b
    shift_msa, scale_msa, gate_msa, shift_mlp, scale_mlp, gate_mlp = \
        np.split(mods, 6)

    d = {}
    d["xT"] = np.ascontiguousarray(x[b, sl].T).reshape(KT, P, TOK)
    d["qkvw8"] = _q8(np.ascontiguousarray(qkv_w.T).reshape(KT, P, 3 * H), SW)
    # attn_out contraction rows regrouped into head pairs:
    # partition p = (h%2)*64 + d, pair index t = h//2
    wt = attn_out_w.T.reshape(6, 2, D, H).transpose(1, 2, 0, 3)
    d["attnw8"] = _q8(np.ascontiguousarray(wt.reshape(P, 6 * H)), SW)
    d["w18"] = _q8(np.ascontiguousarray(mlp_w1.T).reshape(KT, P, Fd), SW)
    d["w2T"] = _q8(np.ascontiguousarray(
        mlp_w2.T.reshape(FT, P, KT, P).transpose(2, 0, 1, 3)), SW)
    d["b1"] = fm(np.asarray(inputs["mlp_b1"], F32))
    d["n1wA"] = fm(np.asarray(inputs["norm1_w"], F32) * (1.0 + scale_msa)) * SX
    d["n2wA"] = fm(np.asarray(inputs["norm2_w"], F32) * (1.0 + scale_mlp)) * SX
    d["sh1"] = fm(shift_msa) * SX
    d["sh2"] = fm(shift_mlp) * SX
    d["g1"] = fm(gate_msa) * DQ
    d["g2dq"] = fm(gate_mlp) * DQ2
    d["b2g"] = fm(np.asarray(inputs["mlp_b2"], F32) * gate_mlp)
    cosT = np.ascontiguousarray(cos[0, sl, 0, 0, :].T)  # [64, 512]
    sinT = np.ascontiguousarray(sin[0, sl, 0, 0, :].T)
    # q/k rotary from scaled psums: q8 = acc*(8*cos/4096) + rot_acc*(8*sin/4096)
    rsc = SQK / (SX * SW)
    d["cos8"] = (np.vstack([cosT, cosT]) * rsc).astype(BF16)
    d["sin8"] = (np.vstack([sinT, sinT]) * rsc).astype(BF16)
    d["rotp"] = _rot_perm().astype(BF16)
    d["onesb"] = np.ones((P, P), BF16)
    bs = np.zeros((P, 2 * P), F32)
    bs[0, 0:D] = 1.0        # even pair slot: heads at partitions 0 / 32
    bs[32, D:P] = 1.0
    bs[64, P:P + D] = 1.0   # odd pair slot: heads at partitions 64 / 96
    bs[96, P + D:2 * P] = 1.0
    d["bsel"] = bs.astype(BF16)
    return d


def kernel(**inputs):
    nc = _get_nc()
    in_maps = [_prep_core_inputs(inputs, c) for c in range(NCORES)]
    res = run_bass_kernel_spmd(nc, in_maps, core_ids=list(range(NCORES)))
    out = np.empty((B, S, H), F32)
    for core in range(NCORES):
        b, j = divmod(core, 4)
        o = res.results[core]["outT"].reshape(H, TOK)
        out[b, j * TOK:(j + 1) * TOK, :] = o.T
    return out


# revision 37
# speedup vs baseline: 1.1359x; 1.1359x over previous
"""DDiT block (adaLN-modulated transformer block) on 8 Trainium2 NeuronCores.

Sharding: tokens split 8 ways (2 batches x 4 sequence chunks of 512 tokens).
Activations kept feature-major ([feature, token]) on-chip. k/v all-gathered
within each batch group of 4 cores in two stages (heads 0-5, then heads
6-11). adaLN modulation (a [B,6H] vector that depends only on c) is folded
into per-core constants on the host, like the weight transposes/quant.

Precision: fp32 residual stream and layernorm statistics; bf16 for LN sums,
rotary and broadcast matmuls; fp8e4 (DoubleRow where the contraction allows)
for the qkv projection, attention scores, attn@v, attn_out, mlp1 and mlp2
matmuls. Scales are powers of two folded into downstream constants, so
dequantization is exact.
"""
import os
import sys

for _p in ("/opt/trn_rl_repo", "/root/.axon_site/_ro/trn_rl_repo"):
    if os.path.isdir(_p) and _p not in sys.path:
        sys.path.append(_p)

import numpy as np
import ml_dtypes

import concourse.bass as bass
import concourse.mybir as mybir
import concourse.tile as tile
from concourse.bass_utils import run_bass_kernel_spmd
from concourse.vector_clock import ScopedClock

BF16 = ml_dtypes.bfloat16
FP8 = ml_dtypes.float8_e4m3
F32 = np.float32

B, S, H, NH, D, Fd = 2, 2048, 768, 12, 64, 3072
P = 128
NCORES = 8
TOK = S // 4            # 512 tokens per core
KT = H // P             # 6 feature tiles of H
FT = Fd // P            # 24 feature tiles of F
VW = D + 4              # 68: v + "32" column (softmax denom) + zero pad,
                        # so the DoubleRow weight AP's chunk stride
                        # (NH*VW = 816 bytes) is 16-byte aligned
EPS = 1e-5

# fp8 scale factors (dequant folded into constants)
SX = 16.0               # xm / xm2 (modulated LN outputs)
SW = 256.0              # all fp8 weights
SQK = 8.0               # nominal rotated q / k scale (see BETA below)
SV = 32.0               # v (also the value of the denominator column)
SO = 16.0               # normalized attention output o
DQ = 1.0 / (SX * SW)    # dequant of fp8 matmul psums (2^-12)
DQ2 = 1.0 / SW          # dequant of the mlp2 psum (hdn is unscaled fp8)
EXP_BIAS = float(-np.log(SQK))  # exp(logit)-scale folded via bias
EXP_SCALE = 1.0 / (D * SQK * SQK) * 8.0  # psum = 64*dot*64 -> logit=dot/8
LOG2E = float(np.log2(np.e))
# sqrt(BETA) is folded into the q/k quantization so the score psum is
# already in "fp8-bits/8" log2 units: the DVE softmax exp needs no multiply.
BETA = EXP_SCALE * LOG2E * 8.0
ESC2 = EXP_SCALE / BETA          # ACT exp scale on the rescaled psum
SCH_C2 = 8.0 * (EXP_BIAS * LOG2E + 7.0 - 0.05730496)
SCH_M = 119.49 - SCH_C2

AF = mybir.ActivationFunctionType
ALU = mybir.AluOpType
DT = mybir.dt
PM = mybir.MatmulPerfMode

RG_BATCH = [[0, 1, 2, 3], [4, 5, 6, 7]]

KBN = P * TOK           # elements of one k tile in an AG buffer
VBN = P * 6 * VW        # elements of one v half-toktile in an AG buffer
AGN = 3 * KBN + 4 * VBN  # per-stage AG payload (fp8 elements)


def _patch_tile_drain():
    """The walrus build here allows at most one semaphore wait on SP
    control instructions; TileContext's exit drain attaches several.
    Split them one-per-NOP."""
    if getattr(tile.TileContext, "_ant_drain_patched", False):
        return

    def _split_multiwaits(nc):
        count = 0
        for f in nc.m.functions:
            for bb in f.blocks:
                insts = bb.instructions
                new = []
                for ins in insts:
                    si = getattr(ins, "sync_info", None)
                    if si is not None and si.on_wait and len(si.on_wait) > 1:
                        waits = list(si.on_wait)
                        si.on_wait = [waits[-1]]
                        for w in waits[:-1]:
                            count += 1
                            nop = mybir.InstNoOp(
                                name=f"antw_{count}_{ins.name}",
                                ins=[], outs=[])
                            nop.engine = ins.engine
                            nop.sync_info = mybir.SyncInfo(
                                on_update=[], on_wait=[w])
                            nc.register_instruction(nop, overwrite=True)
                            new.append(nop)
                    new.append(ins)
                bb.instructions = new

    def _drain_and_barrier(self, tick_clock, wait_clock):
        _split_multiwaits(self.nc)
        drain_inst = self.nc.sync.drain()
        wait_clock.add_sem_waits(
            drain_inst.ins, ScopedClock({None: tick_clock.global_clock})
        )
        si = drain_inst.ins.sync_info
        waits = list(si.on_wait)
        si.on_wait = []
        for w in waits:
            nop = self.nc.sync.nop(nofuse=True, hint="drain_extra_waits")
            nop.ins.sync_info = mybir.SyncInfo(on_update=[], on_wait=[w])
        self.nc.all_engine_barrier()
        popped = self.nc._tile_sem_poison_stack.pop()
        assert popped is self._sem_poison
        self.nc.clear_and_free_semaphores(list(self.sems.allocated().values()))
        self.nc.all_engine_barrier()

    tile.TileContext._drain_and_barrier = _drain_and_barrier
    tile.TileContext._ant_drain_patched = True


def build():
    _patch_tile_drain()
    nc = bass.Bass(num_devices=NCORES)

    def din(name, shape, dt):
        return nc.dram_tensor(name, shape, dt, kind="ExternalInput")

    xT = din("xT", [KT, P, TOK], DT.float32)
    qkvw8 = din("qkvw8", [KT, P, 3 * H], DT.float8e4)
    attnw8 = din("attnw8", [P, 6 * H], DT.float8e4)
    w18 = din("w18", [KT, P, Fd], DT.float8e4)
    w2T = din("w2T", [KT, FT, P, P], DT.float8e4)
    b1 = din("b1", [P, FT], DT.float32)
    n1wA = din("n1wA", [P, KT], DT.float32)   # 16*norm1_w*(1+scale_msa)
    n2wA = din("n2wA", [P, KT], DT.float32)   # 16*norm2_w*(1+scale_mlp)
    sh1 = din("sh1", [P, KT], DT.float32)     # 16*shift_msa
    sh2 = din("sh2", [P, KT], DT.float32)     # 16*shift_mlp
    g1 = din("g1", [P, KT], DT.float32)       # gate_msa*DQ
    g2dq = din("g2dq", [P, KT], DT.float32)   # gate_mlp*DQ2
    b2g = din("b2g", [P, KT], DT.float32)     # mlp_b2*gate_mlp
    cos8 = din("cos8", [P, TOK], DT.bfloat16)   # cos * SQK/(SX*SW)
    sin8 = din("sin8", [P, TOK], DT.bfloat16)   # sin * SQK/(SX*SW)
    rotp = din("rotp", [P, P], DT.bfloat16)
    onesb = din("onesb", [P, P], DT.bfloat16)
    # bsel[r, s*128 + p]: broadcast-select stationary. Head denominators
    # live at partition 32*(h%4); a head pair tt uses slots (0,32) when
    # tt is even (s=0) and (64,96) when odd (s=1); even head -> cols 0-63.
    bsel = din("bsel", [P, 2 * P], DT.bfloat16)

    outT = nc.dram_tensor("outT", [KT, P, TOK], DT.float32,
                          kind="ExternalOutput")

    with tile.TileContext(nc) as tc:
        with tc.tile_pool(name="sb", bufs=1) as sb, \
             tc.tile_pool(name="ps", bufs=1, space="PSUM") as ps, \
             tc.tile_pool(name="dr", bufs=1, space="DRAM") as dr:
            _body(nc, sb, ps, dr, locals())
    return nc


def _body(nc, sb, ps, dr, t):
    xT, qkvw8, attnw8, w18, w2T = t["xT"], t["qkvw8"], t["attnw8"], t["w18"], t["w2T"]
    b1, n1wA, n2wA = t["b1"], t["n1wA"], t["n2wA"]
    sh1, sh2, g1, g2dq, b2g = t["sh1"], t["sh2"], t["g1"], t["g2dq"], t["b2g"]
    cos8, sin8, rotp, onesb, bsel = t["cos8"], t["sin8"], t["rotp"], t["onesb"], t["bsel"]
    outT = t["outT"]

    # x first: everything up to the first AllGather hangs off it
    x_sb = sb.tile([P, KT, TOK], DT.float32)
    for k in range(KT):
        nc.sync.dma_start(x_sb[:, k, :], xT[k])

    # ================= constants ===================================
    zero_c = sb.tile([P, 1], DT.float32)
    nc.vector.memset(zero_c[:], 0.0)
    nc.const_aps.aps[(DT.float32, 0.0)] = zero_c[:]
    eps_c = sb.tile([P, 1], DT.float32)
    nc.vector.memset(eps_c[:], EPS)
    nc.const_aps.aps[(DT.float32, EPS)] = eps_c[:]
    expb_c = sb.tile([P, 1], DT.float32)
    nc.vector.memset(expb_c[:], EXP_BIAS)
    nc.const_aps.aps[(DT.float32, EXP_BIAS)] = expb_c[:]

    # ================= warm-up collective ==========================
    # First collective on the CC stream pays a large arming cost that
    # includes waiting for all peers; fire a 16B dummy immediately so
    # that cost overlaps the compute prologue instead of the k/v AG.
    dummy_sb = sb.tile([1, 4], DT.float32)
    nc.vector.memset(dummy_sb[:], 0.0)
    dummy_in = dr.tile([4], DT.float32)
    dummy_out = dr.tile([4, 4], DT.float32)
    nc.sync.dma_start(dummy_in[:].rearrange("(a b) -> a b", a=1), dummy_sb[:])
    nc.gpsimd.collective_compute(
        "AllGather", ALU.bypass, replica_groups=RG_BATCH,
        ins=[dummy_in[:].opt()], outs=[dummy_out[:].opt()])

    # ================= DMA loads (priority order) ==================
    onesb_sb = sb.tile([P, P], DT.bfloat16)
    nc.sync.dma_start(onesb_sb[:], onesb[:])
    bsel_sb = sb.tile([P, 2 * P], DT.bfloat16)
    nc.sync.dma_start(bsel_sb[:], bsel[:])
    rotp_sb = sb.tile([P, P], DT.bfloat16)
    nc.sync.dma_start(rotp_sb[:], rotp[:])
    cos_sb = sb.tile([P, TOK], DT.bfloat16)
    nc.sync.dma_start(cos_sb[:], cos8[:])
    sin_sb = sb.tile([P, TOK], DT.bfloat16)
    nc.sync.dma_start(sin_sb[:], sin8[:])
    n1wA_sb = sb.tile([P, KT], DT.float32)
    nc.sync.dma_start(n1wA_sb[:], n1wA[:])
    n2wA_sb = sb.tile([P, KT], DT.float32)
    nc.sync.dma_start(n2wA_sb[:], n2wA[:])
    sh1_sb = sb.tile([P, KT], DT.float32)
    nc.sync.dma_start(sh1_sb[:], sh1[:])
    sh2_sb = sb.tile([P, KT], DT.float32)
    nc.sync.dma_start(sh2_sb[:], sh2[:])
    g1_sb = sb.tile([P, KT], DT.float32)
    nc.sync.dma_start(g1_sb[:], g1[:])
    g2dq_sb = sb.tile([P, KT], DT.float32)
    nc.sync.dma_start(g2dq_sb[:], g2dq[:])
    b2g_sb = sb.tile([P, KT], DT.float32)
    nc.sync.dma_start(b2g_sb[:], b2g[:])

    # qkv weights: k columns first (gate the first AG)
    qkvw_sb = sb.tile([P, KT, 3 * H], DT.float8e4)
    for k in range(KT):
        nc.sync.dma_start(qkvw_sb[:, k, H:2 * H], qkvw8[k][:, H:2 * H])
    for k in range(KT):
        nc.sync.dma_start(qkvw_sb[:, k, 2 * H:3 * H], qkvw8[k][:, 2 * H:3 * H])
    for k in range(KT):
        nc.sync.dma_start(qkvw_sb[:, k, 0:H], qkvw8[k][:, 0:H])
    attnw_sb = sb.tile([P, 6, H], DT.float8e4)
    nc.sync.dma_start(attnw_sb[:].rearrange("p a b -> p (a b)"), attnw8[:])
    b1_sb = sb.tile([P, FT], DT.float32)
    nc.sync.dma_start(b1_sb[:], b1[:])
    w1_sb = sb.tile([P, KT, Fd], DT.float8e4)
    for k in range(KT):
        nc.sync.dma_start(w1_sb[:, k, :], w18[k])

    f32s = dict(tag="f32s", bufs=3)

    def layer_norm(src_sb, A_tile, sh_tile, xm_out):
        """src [128,KT,TOK] f32 -> xm_out [128,KT,TOK] fp8e4
        = 16*(modulated LN output)."""
        s_ps = ps.tile([1, TOK], DT.float32, tag="duo", bufs=3, name="s_ps")
        q_ps = ps.tile([1, TOK], DT.float32, tag="duo", bufs=3, name="q_ps")
        onesf_col = sb.tile([P, 1], DT.float32, tag="of", bufs=1, name="of")
        nc.vector.memset(onesf_col[:], 1.0)
        for tt in range(KT):
            xsq = sb.tile([P, TOK], DT.bfloat16, tag="sq", bufs=2, name="xsq")
            nc.gpsimd.tensor_tensor(xsq[:], src_sb[:, tt, :], src_sb[:, tt, :],
                                    ALU.mult)
            nc.tensor.matmul(s_ps[:], onesf_col[:], src_sb[:, tt, :],
                             start=(tt == 0), stop=(tt == KT - 1))
            nc.tensor.matmul(q_ps[:], onesb_sb[:, 0:1], xsq[:],
                             start=(tt == 0), stop=(tt == KT - 1))
        sa = sb.tile([1, TOK], DT.float32, tag="st_a", bufs=1, name="sa")
        sb2 = sb.tile([1, TOK], DT.float32, tag="st_b", bufs=1, name="sb2")
        sc_ = sb.tile([1, TOK], DT.float32, tag="st_c", bufs=1, name="sc_")
        nc.vector.tensor_scalar(sa[:], s_ps[:], 1.0 / H, None, ALU.mult)
        nc.vector.tensor_scalar(sb2[:], q_ps[:], 1.0 / H, None, ALU.mult)
        nc.vector.tensor_tensor(sc_[:], sa[:], sa[:], ALU.mult)
        nc.vector.tensor_tensor(sb2[:], sb2[:], sc_[:], ALU.subtract)
        nc.scalar.activation(sc_[:], sb2[:], AF.Sqrt, bias=EPS)
        nc.vector.reciprocal(sb2[:], sc_[:])   # sb2 = rstd (f32)
        rstd16 = sb.tile([1, TOK], DT.bfloat16, tag="st_d", bufs=1,
                         name="rstd16")
        nc.vector.tensor_scalar(rstd16[:], sb2[:], 1.0, None, ALU.mult)
        mr16 = sb.tile([1, TOK], DT.bfloat16, tag="st_e", bufs=1, name="mr16")
        nc.vector.tensor_tensor(mr16[:], sa[:], sb2[:], ALU.mult)
        rstd_ps = ps.tile([P, TOK], DT.float32, tag="duo", bufs=3,
                          name="rstd_ps")
        mr_ps = ps.tile([P, TOK], DT.float32, tag="duo", bufs=3,
                        name="mr_ps")
        nc.tensor.matmul(rstd_ps[:], onesb_sb[0:1, :], rstd16[:],
                         start=True, stop=True)
        nc.tensor.matmul(mr_ps[:], onesb_sb[0:1, :], mr16[:],
                         start=True, stop=True)
        for tt in range(KT):
            t1 = sb.tile([P, TOK], DT.float32, **f32s, name="t1")
            nc.vector.tensor_tensor(t1[:], src_sb[:, tt, :], rstd_ps[:],
                                    ALU.mult)
            nc.vector.tensor_tensor(t1[:], t1[:], mr_ps[:], ALU.subtract)
            nc.vector.tensor_scalar(
                xm_out[:, tt, :], t1[:], A_tile[:, tt:tt + 1],
                sh_tile[:, tt:tt + 1], ALU.mult, ALU.add)

    # ================= LN1 + qkv ===================================
    xm_sb = sb.tile([P, KT, TOK], DT.float8e4, tag="xm")
    layer_norm(x_sb, n1wA_sb, sh1_sb, xm_sb)

    agA_in = dr.tile([AGN], DT.float8e4)
    agA_out = dr.tile([4, AGN], DT.float8e4)
    agB_in = dr.tile([AGN], DT.float8e4)
    agB_out = dr.tile([4, AGN], DT.float8e4)

    kfull_sb = sb.tile([P, KT, S], DT.float8e4)
    vfull_sb = sb.tile([P, 16, NH * VW], DT.float8e4)

    def qk_tile(m, dest, dest2=None, ve=None):
        """Feature tile m of the qkv projection + rotary -> fp8 (x8).
        ve: engine for the SBUF-only rotary combine (Pool for k tiles,
        emitted before the collective triggers on the Pool queue; DVE
        for q tiles). The PSUM-reading multiply always runs on DVE."""
        acc = ps.tile([P, TOK], DT.float32, tag="duo", bufs=3, name="qk_acc")
        for k in range(0, KT, 2):
            nc.tensor.matmul(acc[:], qkvw_sb[:, k:k + 2, m * P:(m + 1) * P],
                             xm_sb[:, k:k + 2, :],
                             start=(k == 0), stop=(k == KT - 2),
                             perf_mode=PM.DoubleRow)
        pre = sb.tile([P, TOK], DT.bfloat16, tag="qpre", bufs=2, name="pre")
        nc.scalar.copy(pre[:], acc[:])
        rot = ps.tile([P, TOK], DT.float32, tag="duo", bufs=3, name="rot")
        nc.tensor.matmul(rot[:], rotp_sb[:], pre[:], start=True, stop=True)
        r1 = sb.tile([P, TOK], DT.bfloat16, tag="rr1", bufs=2, name="r1")
        nc.gpsimd.tensor_tensor(r1[:], pre[:], cos_sb[:], ALU.mult)
        r2 = sb.tile([P, TOK], DT.bfloat16, tag="rr2", bufs=2, name="r2")
        nc.vector.tensor_tensor(r2[:], rot[:], sin_sb[:], ALU.mult)
        if dest2 is None:
            ve.tensor_tensor(dest, r1[:], r2[:], ALU.add)
        else:
            ve.tensor_tensor(dest, r1[0:D, :], r2[0:D, :], ALU.add)
            ve.tensor_tensor(dest2, r1[D:P, :], r2[D:P, :], ALU.add)

    vaug_sb = sb.tile([P, 4, NH * VW], DT.float8e4)
    for tt in range(4):
        nc.vector.memset(
            vaug_sb[:, tt, :].rearrange("p (h w) -> p h w", w=VW)[:, :, D:D + 1],
            SV)
        nc.vector.memset(
            vaug_sb[:, tt, :].rearrange("p (h w) -> p h w", w=VW)[:, :, D + 1:VW],
            0.0)

    def v_tile(tt, half):
        acc = ps.tile([P, 6 * D], DT.float32, tag="duo", bufs=3,
                      name="v_acc")
        for k in range(0, KT, 2):
            nc.tensor.matmul(
                acc[:], xm_sb[:, k:k + 2, tt * P:(tt + 1) * P],
                qkvw_sb[:, k:k + 2, 2 * H + half * 6 * D:
                        2 * H + (half + 1) * 6 * D],
                start=(k == 0), stop=(k == KT - 2),
                perf_mode=PM.DoubleRow)
        nc.scalar.mul(
            vaug_sb[:, tt, :]
            .rearrange("p (h w) -> p h w", w=VW)[:, half * 6:(half + 1) * 6, 0:D],
            acc[:].rearrange("p (h d) -> p h d", d=D), SV * DQ)

    def ag(in_t, out_t):
        nc.gpsimd.collective_compute(
            "AllGather", ALU.bypass, replica_groups=RG_BATCH,
            ins=[in_t[:].opt()], outs=[out_t[:].opt()])

    # ---- stage A inputs: k feature tiles 0-2 + v heads 0-5 --------
    for mm_ in range(3):
        kt_t = sb.tile([P, TOK], DT.float8e4, tag="ktmp", bufs=2,
                       name=f"ktmpA_{mm_}")
        qk_tile(KT + mm_, kt_t[:], ve=nc.gpsimd)
        nc.sync.dma_start(
            agA_in[mm_ * KBN:(mm_ + 1) * KBN].rearrange("(p s) -> p s", p=P),
            kt_t[:])
    for tt in range(4):
        v_tile(tt, 0)
        nc.sync.dma_start(
            agA_in[3 * KBN + tt * VBN:3 * KBN + (tt + 1) * VBN]
            .rearrange("(p w) -> p w", p=P),
            vaug_sb[:, tt, 0:6 * VW])
    ag(agA_in, agA_out)

    # ---- stage B inputs: k feature tiles 3-5 + v heads 6-11 -------
    for mm_ in range(3):
        kt_t = sb.tile([P, TOK], DT.float8e4, tag="ktmp", bufs=2,
                       name=f"ktmpB_{mm_}")
        qk_tile(KT + 3 + mm_, kt_t[:], ve=nc.gpsimd)
        nc.sync.dma_start(
            agB_in[mm_ * KBN:(mm_ + 1) * KBN].rearrange("(p s) -> p s", p=P),
            kt_t[:])
    for tt in range(4):
        v_tile(tt, 1)
        nc.sync.dma_start(
            agB_in[3 * KBN + tt * VBN:3 * KBN + (tt + 1) * VBN]
            .rearrange("(p w) -> p w", p=P),
            vaug_sb[:, tt, 6 * VW:12 * VW])
    ag(agB_in, agB_out)

    # ---- unpack gathered k/v (ahead of q: fires the moment AGs land)
    for r in range(4):
        nc.sync.dma_start(
            kfull_sb[:, 0:3, r * TOK:(r + 1) * TOK],
            agA_out[r, 0:3 * KBN].rearrange("(k p s) -> p k s", p=P, s=TOK))
        nc.sync.dma_start(
            vfull_sb[:, 4 * r:4 * (r + 1), 0:6 * VW],
            agA_out[r, 3 * KBN:].rearrange("(t p w) -> p t w", p=P, w=6 * VW))
    for r in range(4):
        nc.sync.dma_start(
            kfull_sb[:, 3:6, r * TOK:(r + 1) * TOK],
            agB_out[r, 0:3 * KBN].rearrange("(k p s) -> p k s", p=P, s=TOK))
        nc.sync.dma_start(
            vfull_sb[:, 4 * r:4 * (r + 1), 6 * VW:12 * VW],
            agB_out[r, 3 * KBN:].rearrange("(t p w) -> p t w", p=P, w=6 * VW))

    # ---- mlp2 weights prefetched during attention -----------------
    w2all = sb.tile([P, KT, FT, P], DT.float8e4, name="w2all")
    for m in range(KT):
        nc.sync.dma_start(w2all[:, m, :, :], w2T[m].rearrange("k p q -> p k q"))

    # ---- q while the all-gathers are in flight --------------------
    q_sb = sb.tile([P, KT, TOK], DT.float8e4, name="q8")
    for m in range(KT):
        qk_tile(m, q_sb[:, m, :], ve=nc.vector)

    # ---- PE warm-up filler ----------------------------------------
    # The AG-A wait would otherwise idle the PE past the HAM MID window,
    # so attention would start clock-throttled to 1.2 GHz and the first
    # idle-triggered re-throttle tends to stick. Burn the wait with
    # discarded matmuls that keep the activity monitor in the 8/8 state.
    for i in range(30):
        fl = ps.tile([P, TOK], DT.float32, tag="duo", bufs=3,
                     name=f"fill_{i}")
        nc.tensor.matmul(fl[:], rotp_sb[:], cos_sb[:], start=True, stop=True)

    # ================= attention ===================================
    o8_sb = sb.tile([P, 6, TOK], DT.float8e4, name="o8")
    o_raw = sb.tile([P, 6, TOK], DT.bfloat16, name="o_raw")
    # head h's denominator at partition 32*(h%4), column group h//4; the
    # unused partitions are preset to 1.0 so the batched reciprocal and the
    # broadcast matmul never see junk (0 * NaN).
    denoms = sb.tile([P, 3, TOK], DT.float32, name="denoms")
    nc.vector.memset(denoms[:].rearrange("p a s -> p (a s)"), 1.0)
    rd_f = sb.tile([P, 3, TOK], DT.float32, name="rd_f")
    rd16 = sb.tile([P, 3, TOK], DT.bfloat16, name="rd16")

    def emit_scores(h, pair, eq, dve_exp):
        ht = h // 2
        ro = (h % 2) * D
        sc = ps.tile([P, 2 * TOK], DT.float32, tag="duo", bufs=3,
                     name=f"sc_{h}_{pair}")
        for i in range(2):
            kj = 2 * pair + i
            nc.tensor.matmul(
                sc[:, i * TOK:(i + 1) * TOK],
                kfull_sb[ro:ro + D, ht, kj * P:(kj + 1) * P],
                q_sb[ro:ro + D, ht, :],
                start=True, stop=True, tile_position=(ro, 0))
        dst = eq[:, (pair % 2) * 2:(pair % 2) * 2 + 2, :].rearrange(
            "p a s -> p (a s)")
        if dve_exp:
            # Schraudolph exp straight into fp8 bits: psum is pre-scaled to
            # log2 units x8 (BETA folded into q/k), so one min+add writes
            # uint8 = clamp(p + C2, 0, 126); negatives saturate to 0 and the
            # min caps at 0x77 = 240 (IEEE e4m3: exp-15 bit patterns are inf/NaN).
            nc.vector.tensor_scalar(dst.bitcast(DT.uint8), sc[:],
                                    SCH_M, SCH_C2, ALU.min, ALU.add)
        else:
            nc.scalar.activation(dst, sc[:], AF.Exp, scale=ESC2,
                                 bias=EXP_BIAS)

    def emit_av(h, pair, eq, o_ps, start, stop):
        nc.tensor.matmul(
            o_ps[:], vfull_sb[:, 2 * pair:2 * pair + 2, h * VW:(h + 1) * VW],
            eq[:, (pair % 2) * 2:(pair % 2) * 2 + 2, :],
            start=start, stop=stop, perf_mode=PM.DoubleRow)

    def save_head(h, o_ps):
        # raw (unnormalized) o and its denominator row; normalization is
        # batched per 4 heads so the DVE reciprocal runs 3x, not 12x.
        ro = (h % 2) * D
        tt = h // 2
        dslot = 32 * (h % 4)
        if h % 2:
            nc.vector.tensor_copy(o_raw[ro:ro + D, tt, :], o_ps[0:D, :])
            nc.vector.tensor_copy(denoms[dslot:dslot + 1, h // 4, :],
                                  o_ps[D:D + 1, :])
        else:
            nc.scalar.copy(o_raw[ro:ro + D, tt, :], o_ps[0:D, :])
            nc.scalar.copy(denoms[dslot:dslot + 1, h // 4, :],
                           o_ps[D:D + 1, :])

    def recip_chunk(g, c):
        # 1/4 of the batched reciprocal for heads 4g..4g+3; chunked so the
        # DVE queue can interleave attention exps between pieces
        nc.vector.reciprocal(rd_f[:, g, c * 128:(c + 1) * 128],
                             denoms[:, g, c * 128:(c + 1) * 128])
        if c == 3:
            nc.vector.tensor_scalar(rd16[:, g, :], rd_f[:, g, :], SO, None,
                                    ALU.mult)

    def norm_pair(tt):
        # heads 2tt, 2tt+1 -> o8 = o_raw * SO/denom
        rdb = ps.tile([P, TOK], DT.float32, tag="duo", bufs=3,
                      name=f"rdb_{tt}")
        nc.tensor.matmul(rdb[:], bsel_sb[:, (tt % 2) * P:(tt % 2 + 1) * P],
                         rd16[:, tt // 2, :], start=True, stop=True)
        nc.vector.tensor_tensor(o8_sb[:, tt, :], o_raw[:, tt, :],
                                rdb[:], ALU.mult)

    # Software-pipelined head loop: scores run 2 pairs ahead of attn@v so
    # the tensor queue never waits on a just-issued exp; exps are split
    # between the scalar engine (table exp) and DVE (Schraudolph bf16 exp
    # via int16 bit trick + clamped fp8 cast) to keep either from pacing
    # the PE. On heads that also carry the batched-reciprocal DVE work,
    # the DVE gets one pair less.
    eq_map = {}
    o_tiles = {}

    def emit_sc_exp(h, pair):
        if pair % 2 == 0:
            eq_map[(h, pair // 2)] = sb.tile(
                [P, 4, TOK], DT.float8e4, tag="scr4", bufs=6,
                name=f"exp_{h}_{pair // 2}")
        eq = eq_map[(h, pair // 2)]
        carries_recip = h % 4 == 0 and h > 0
        dve_pairs = (1, 4) if carries_recip else (1, 4, 6)
        emit_scores(h, pair, eq, pair in dve_pairs)
        if carries_recip and pair in (2, 3, 5, 7):
            recip_chunk(h // 4 - 1, {2: 0, 3: 1, 5: 2, 7: 3}[pair])

    def emit_av_for(h, pair):
        if pair == 0:
            o_tiles[h] = ps.tile([VW, TOK], DT.float32, tag="oo", bufs=2,
                                 name=f"o_ps_{h}")
        emit_av(h, pair, eq_map[(h, pair // 2)], o_tiles[h],
                start=(pair == 0), stop=(pair == 7))
        if pair == 7:
            for q_ in range(4):
                eq_map.pop((h, q_), None)
            save_head(h, o_tiles.pop(h))
            if h % 4 == 3 and h // 4 >= 1:
                # group g-1's broadcasts: its reciprocal is ~4 heads old,
                # so the rdb matmul never blocks the PE queue
                g = h // 4
                norm_pair(2 * g - 2)
                norm_pair(2 * g - 1)

    pend = []
    for h in range(NH):
        for pair in range(8):
            emit_sc_exp(h, pair)
            pend.append((h, pair))
            if len(pend) > 2:
                emit_av_for(*pend.pop(0))
    while pend:
        emit_av_for(*pend.pop(0))

    # ================= attn_out + residual =========================
    # Head-pair groups 0-3 are normalized by now; run their attn_out
    # accumulation while the DVE computes the last reciprocal batch, so
    # the tail chain only gates the final third of the contraction.
    # Six [P,TOK] accumulators live as halves of three 2-bank slots.
    ao_accs = [ps.tile([P, 2 * TOK], DT.float32, tag="duo", bufs=3,
                       name=f"ao_{i}") for i in range(3)]

    def ao_acc(m):
        return ao_accs[m // 2][:, (m % 2) * TOK:(m % 2 + 1) * TOK]

    for m in range(KT):
        for q_ in range(2):
            nc.tensor.matmul(
                ao_acc(m), attnw_sb[:, 2 * q_:2 * q_ + 2, m * P:(m + 1) * P],
                o8_sb[:, 2 * q_:2 * q_ + 2, :],
                start=(q_ == 0), stop=False, perf_mode=PM.DoubleRow)

    for c in range(4):
        recip_chunk(2, c)
    norm_pair(4)
    norm_pair(5)

    for m in range(KT):
        nc.tensor.matmul(
            ao_acc(m), attnw_sb[:, 4:6, m * P:(m + 1) * P],
            o8_sb[:, 4:6, :],
            start=False, stop=True, perf_mode=PM.DoubleRow)
        tg = sb.tile([P, TOK], DT.float32, **f32s, name="tg")
        nc.vector.tensor_scalar(tg[:], ao_acc(m), g1_sb[:, m:m + 1], None,
                                ALU.mult)
        nc.vector.tensor_tensor(x_sb[:, m, :], tg[:], x_sb[:, m, :], ALU.add)

    # ================= LN2 + MLP ===================================
    xm2_sb = sb.tile([P, KT, TOK], DT.float8e4, tag="xm", name="xm2")
    layer_norm(x_sb, n2wA_sb, sh2_sb, xm2_sb)

    hdn_tiles = []
    for g in range(KT):
        hq = sb.tile([P, 4, TOK], DT.float8e4, tag="hdn", bufs=6,
                     name=f"hdn_{g}")
        hdn_tiles.append(hq)
        for r in range(4):
            m = g * 4 + r
            acc = ps.tile([P, TOK], DT.float32, tag="duo", bufs=3,
                          name=f"m1_{m}")
            for k in range(0, KT, 2):
                nc.tensor.matmul(acc[:], w1_sb[:, k:k + 2, m * P:(m + 1) * P],
                                 xm2_sb[:, k:k + 2, :],
                                 start=(k == 0), stop=(k == KT - 2),
                                 perf_mode=PM.DoubleRow)
            nc.scalar.activation(hq[:, r, :], acc[:], AF.Gelu_apprx_tanh,
                                 bias=b1_sb[:, m:m + 1], scale=DQ)

    for m in range(KT):
        acc = ps.tile([P, TOK], DT.float32, tag="duo", bufs=3,
                      name=f"m2_{m}")
        for k in range(0, FT, 2):
            nc.tensor.matmul(acc[:], w2all[:, m, k:k + 2, :],
                             hdn_tiles[k // 4][:, k % 4:k % 4 + 2, :],
                             start=(k == 0), stop=(k == FT - 2),
                             perf_mode=PM.DoubleRow)
        tg = sb.tile([P, TOK], DT.float32, **f32s, name="tg2")
        nc.vector.tensor_scalar(tg[:], acc[:], g2dq_sb[:, m:m + 1],
                                b2g_sb[:, m:m + 1], ALU.mult, ALU.add)
        nc.vector.tensor_tensor(x_sb[:, m, :], tg[:], x_sb[:, m, :], ALU.add)
        nc.sync.dma_start(outT[m], x_sb[:, m, :])


_CACHE = {}


def _get_nc():
    if "nc" not in _CACHE:
        _CACHE["nc"] = build()
    return _CACHE["nc"]


def _rot_perm():
    blk = np.zeros((D, D), F32)
    for i in range(32):
        blk[i, i + 32] = 1.0
    for i in range(32, D):
        blk[i, i - 32] = -1.0
    out = np.zeros((P, P), F32)
    out[0:D, 0:D] = blk
    out[D:P, D:P] = blk
    return out


def _q8(w, s):
    return np.clip(np.asarray(w, F32) * s, -240.0, 240.0).astype(FP8)


def _prep_core_inputs(inputs, core):
    b, j = divmod(core, 4)
    sl = slice(j * TOK, (j + 1) * TOK)
    x = np.asarray(inputs["x"], F32)
    qkv_w = np.asarray(inputs["qkv_w"], F32)
    attn_out_w = np.asarray(inputs["attn_out_w"], F32)
    mlp_w1 = np.asarray(inputs["mlp_w1"], F32)
    mlp_w2 = np.asarray(inputs["mlp_w2"], F32)
    ada_w = np.asarray(inputs["ada_w"], F32)
    ada_b = np.asarray(inputs["ada_b"], F32)
    cc = np.asarray(inputs["c"], F32)
    cos = np.asarray(inputs["cos"], F32)
    sin = np.asarray(inputs["sin"], F32)

    def fm(vec):  # [n*128] -> [128, n] feature-major
        return np.ascontiguousarray(vec.reshape(-1, P).T, dtype=F32)

    # adaLN modulation: a [6H] vector depending only on c[b]; fold on host
    mods = cc[b] @ ada_w.T + ada_b
    shift_msa, scale_msa, gate_msa, shift_mlp, scale_mlp, gate_mlp = \
        np.split(mods, 6)

    d = {}
    d["xT"] = np.ascontiguousarray(x[b, sl].T).reshape(KT, P, TOK)
    d["qkvw8"] = _q8(np.ascontiguousarray(qkv_w.T).reshape(KT, P, 3 * H), SW)
    # attn_out contraction rows regrouped into head pairs:
    # partition p = (h%2)*64 + d, pair index t = h//2
    wt = attn_out_w.T.reshape(6, 2, D, H).transpose(1, 2, 0, 3)
    d["attnw8"] = _q8(np.ascontiguousarray(wt.reshape(P, 6 * H)), SW)
    d["w18"] = _q8(np.ascontiguousarray(mlp_w1.T).reshape(KT, P, Fd), SW)
    d["w2T"] = _q8(np.ascontiguousarray(
        mlp_w2.T.reshape(FT, P, KT, P).transpose(2, 0, 1, 3)), SW)
    d["b1"] = fm(np.asarray(inputs["mlp_b1"], F32))
    d["n1wA"] = fm(np.asarray(inputs["norm1_w"], F32) * (1.0 + scale_msa)) * SX
    d["n2wA"] = fm(np.asarray(inputs["norm2_w"], F32) * (1.0 + scale_mlp)) * SX
    d["sh1"] = fm(shift_msa) * SX
    d["sh2"] = fm(shift_mlp) * SX
    d["g1"] = fm(gate_msa) * DQ
    d["g2dq"] = fm(gate_mlp) * DQ2
    d["b2g"] = fm(np.asarray(inputs["mlp_b2"], F32) * gate_mlp)
    cosT = np.ascontiguousarray(cos[0, sl, 0, 0, :].T)  # [64, 512]
    sinT = np.ascontiguousarray(sin[0, sl, 0, 0, :].T)
    # q/k rotary from scaled psums; sqrt(BETA) folds the softmax exp input
    # scale into the quantization so score psums are in log2-bits/8 units
    rsc = SQK / (SX * SW) * float(np.sqrt(BETA))
    d["cos8"] = (np.vstack([cosT, cosT]) * rsc).astype(BF16)
    d["sin8"] = (np.vstack([sinT, sinT]) * rsc).astype(BF16)
    d["rotp"] = _rot_perm().astype(BF16)
    d["onesb"] = np.ones((P, P), BF16)
    bs = np.zeros((P, 2 * P), F32)
    bs[0, 0:D] = 1.0        # even pair slot: heads at partitions 0 / 32
    bs[32, D:P] = 1.0
    bs[64, P:P + D] = 1.0   # odd pair slot: heads at partitions 64 / 96
    bs[96, P + D:2 * P] = 1.0
    d["bsel"] = bs.astype(BF16)
    return d


def kernel(**inputs):
    nc = _get_nc()
    in_maps = [_prep_core_inputs(inputs, c) for c in range(NCORES)]
    res = run_bass_kernel_spmd(nc, in_maps, core_ids=list(range(NCORES)))
    out = np.empty((B, S, H), F32)
    for core in range(NCORES):
        b, j = divmod(core, 4)
        o = res.results[core]["outT"].reshape(H, TOK)
        out[b, j * TOK:(j + 1) * TOK, :] = o.T
    return out
